# revision 25
# baseline (speedup 1.0000x reference)
"""GRU sequence model kernel for Trainium2 (8 NeuronCores, data-parallel).

Math (per reference):
  u  = x @ W_in.T + b_in              [B,T,H]
  ig = u @ W_ih.T + b_ih              [B,T,3H]   (folded: ig = x@W_c.T + b_c,
                                       with b_c as an extra K-row of the GEMM)
  scan over T:  hg = h @ W_hh.T
                r = sig(ig_r+hg_r); z = sig(ig_z+hg_z)
                n = tanh(ig_n + r*(hg_n + b_n)); h' = n + z*(h-n)
  out = h_T @ W_out.T + b_out         [B,OUT]

Sharding: B=256 split 32/core across 8 cores; weights replicated; T scan local.

Device layout is feature-on-partitions ("transposed"):
  state  hT  [128, 2, BL] f32   (h chunk c*128.., BL batch)
  psum   P_r/P_z/P_n [128, 2, BL] preloaded with ig_rz via identity matmuls /
         b_n via a K=2 selector matmul, then accumulating W_hh matmuls.

Scan recurrence is split-state: h' = w + nzc with w = z*h (available right
after Sig(z), early in the step) and nzc = (1-z)*n (available only after
tanh).  All W_hh matmuls accumulate the two addends separately, so the
w-pass runs during the tanh window and only the nzc-pass (12 small matmuls)
sits on the serial critical path:
  nzc_b -> PE nzc-pass (r group first) -> Sig(r) -> t2=r*P_n -> npre ->
  tanh -> nzc_b
"""

import sys

sys.path.insert(0, "/opt/trn_rl_repo")

import numpy as np

import concourse.bacc as bacc
import concourse.tile as tile
from concourse import mybir
from concourse.bass_utils import run_bass_kernel_spmd

# ---------------------------------------------------------------------------
# Custom DVE ops: polynomial sigmoid/tanh so the whole scan chain runs on the
# DVE with no ACT round-trips (each ACT op costs ~450ns: 185ns SBUF-access
# bubble in busy time plus another 185ns before its semaphore fires).
# Registered through the documented dve_ops extension point (OPS +
# _SUB_OPCODE_FOR_NAME + CUSTOM_DVE_SPECS) with shas computed at import.
# ---------------------------------------------------------------------------
from concourse import dve_ops as _dvo
from concourse.dve_spec import C0, C1, C2, One, Spec, Src0, Src1, Zero, maxx, minn, sq
from concourse.dve_table_gen import dve_ver_for
from concourse.dve_uop import DveOpSpec
from concourse.dve_spec import _has_src1 as _spec_has_src1, lower as _spec_lower


def _register_dve_op(name, spec, subdim=False):
    if name in _dvo._SUB_OPCODE_FOR_NAME:
        return next(op for op in _dvo.OPS if op.name == name)
    ver = dve_ver_for("TRN2")
    row = _dvo._CUSTOM_DVE_ROW_BASE + len(_dvo.OPS)
    lowered = DveOpSpec(
        name=name, opcode=row, uops=_spec_lower(spec, ver=ver),
        rd1_en=_spec_has_src1(spec),
    )
    op = _dvo.DveOp(name, spec, subdim=subdim, uops_sha={ver: lowered.sha(ver)})
    _dvo.OPS.append(op)
    _dvo._SUB_OPCODE_FOR_NAME[name] = row
    _dvo.CUSTOM_DVE_SPECS[name] = spec
    return op


def _mk_ops():
    # out = in0*(c0 + u*(c1 + u*c2)), u = in0^2  — odd quintic
    _u0 = sq(Src0)
    ODD5 = _register_dve_op(
        "GRU_ODD5_ANT",
        Spec(
            body=Src0 * (C0 + _u0 * (C1 + _u0 * C2)),
            reference=lambda in0, in1, c0, c1, c2: (
                lambda u: in0 * (c0 + u * (c1 + u * c2))
            )(in0 * in0),
        ),
    )
    # out = x*(c0 + u*(c1 + u*c2)), x = in0+in1, u = x^2 — odd quintic of a sum
    _x = Src0 + Src1
    _ux = sq(_x)
    ODD5S = _register_dve_op(
        "GRU_ODD5SUM_ANT",
        Spec(
            body=_x * (C0 + _ux * (C1 + _ux * C2)),
            reference=lambda in0, in1, c0, c1, c2: (
                lambda x: (lambda u: x * (c0 + u * (c1 + u * c2)))(x * x)
            )(in0 + in1),
        ),
    )
    # out = clip(in0 + c2, 0, 1) * in1  — sigmoid shift/clamp fused with mul
    SIGCM = _register_dve_op(
        "GRU_SIGCM_ANT",
        Spec(
            body=maxx(minn(Src0 + C2, One), Zero) * Src1,
            reference=lambda in0, in1, c0, c1, c2: np.clip(in0 + c2, 0.0, 1.0) * in1,
        ),
    )
    # out = in0 * clip(in1, c1, c0)  — tanh clamp fused with mul; the clamped
    # operand is Src1 so the zc factor (early, cross-engine) sits in Src0 and
    # its wait lands on the EventSemaphore while the late same-engine dep
    # rides the attached wait slot.
    TANHCM = _register_dve_op(
        "GRU_TANHCM_ANT",
        Spec(
            body=Src0 * maxx(minn(Src1, C0), C1),
            reference=lambda in0, in1, c0, c1, c2: in0 * np.minimum(
                np.maximum(in1, c1), c0
            ),
        ),
    )
    return ODD5, ODD5S, SIGCM, TANHCM


ODD5_OP, ODD5S_OP, SIGCM_OP, TANHCM_OP = _mk_ops()

# polynomial constants (fit against the reference value ranges; see notes)
SIG5 = (0.2460386709429935, -0.016334294476830553, 0.0006460262005284434)
T4C = (0.24960876935340268, -0.004928416054628486, 7.916056459559794e-05)
D1C = (1.988672907492938, -1.7884419833362064, 0.9587148526678364)
D2C = (1.975424316113866, -1.6426980934515871, 0.6994047528181397)

B, T, IN, H, OUT = 256, 2048, 64, 256, 32
N_CORES = 8
BL = B // N_CORES  # 32 batch rows per core
TC = 64  # scan chunk length (steps per ig buffer)
G3 = 3 * H
F32 = mybir.dt.float32
BF16 = mybir.dt.bfloat16

_nc_cache = {}


def _emit(ctx, tc, aps, T_total):
    nc = tc.nc
    n_chunks = T_total // TC
    Sig = mybir.ActivationFunctionType.Sigmoid
    Tanh = mybir.ActivationFunctionType.Tanh
    Mult = mybir.AluOpType.mult
    Add = mybir.AluOpType.add

    singles = ctx.enter_context(tc.tile_pool(name="singles", bufs=1))
    xpool = ctx.enter_context(tc.tile_pool(name="xpool", bufs=2))
    igpool = ctx.enter_context(tc.tile_pool(name="igpool", bufs=2))
    ew = ctx.enter_context(tc.tile_pool(name="ew", bufs=16))
    state = ctx.enter_context(tc.tile_pool(name="state", bufs=4))
    prz = ctx.enter_context(tc.tile_pool(name="prz", bufs=2, space="PSUM"))
    pn = ctx.enter_context(tc.tile_pool(name="pn", bufs=2, space="PSUM"))
    pgemm = ctx.enter_context(tc.tile_pool(name="pgemm", bufs=2, space="PSUM"))

    # ---- weights into SBUF (once) ----
    whh_sb = singles.tile([128, 2, G3], BF16)  # [k, kc, g] : W_hh.T chunks
    nc.sync.dma_start(out=whh_sb, in_=aps["whhT"].rearrange("(c k) g -> k c g", k=128))
    wc_sb = singles.tile([IN + 1, G3], BF16)  # W_c.T with b_c as row IN
    nc.sync.dma_start(out=wc_sb, in_=aps["wcT"])
    bnl_sb = singles.tile([2, 128], BF16)  # b_n chunks as K=2 matmul lhsT
    nc.sync.dma_start(out=bnl_sb, in_=aps["bnl"])
    sel_sb = singles.tile([2, 2, BL], BF16)  # block selector rhs
    nc.sync.dma_start(out=sel_sb, in_=aps["sel"])
    ident = singles.tile([128, 128], BF16)
    nc.sync.dma_start(out=ident, in_=aps["ident"])
    wo_sb = singles.tile([128, 2, OUT], F32)  # W_out.T chunks [k, kc, o]
    nc.sync.dma_start(out=wo_sb, in_=aps["woT"].rearrange("(c k) o -> k c o", k=128))
    bo_sb = singles.tile([OUT, 1], F32)
    nc.sync.dma_start(out=bo_sb, in_=aps["bob"])

    # ---- state ----
    hT = [state.tile([128, 2, BL], F32, tag="h32", name="hT0")]
    nc.vector.memset(hT[0], 0.0)
    w_b0 = state.tile([128, 2, BL], BF16, tag="wb0", name="wb0")
    nc.vector.memset(w_b0, 0.0)  # w(t=0) = z*h(-1) = 0

    xT = aps["xT"]  # [IN, T_total, BL]

    def load_x(c):
        # row IN is the constant-1 row that multiplies the b_c row of wc_sb;
        # pool slots rotate round-robin so only the first bufs chunks memset it.
        xc = xpool.tile([IN + 1, TC * BL], BF16, tag="xc")
        nc.sync.dma_start(
            out=xc[0:IN, :],
            in_=xT[:, c * TC : (c + 1) * TC, :].rearrange("i t b -> i (t b)"),
        )
        if c < 2:
            nc.vector.memset(xc[IN : IN + 1, :], 1.0)
        return xc

    def gemm_ig(c, xc):
        """igbuf[p, t, gc*BL+b] = (W_c @ x + b_c)[g, t, b]"""
        igbuf = igpool.tile([128, TC, 6 * BL], BF16, tag="ig")
        nblk = TC * BL // 512
        for nb in range(nblk):
            for gc in range(6):
                pg = pgemm.tile([128, 512], F32, tag="pg")
                nc.tensor.matmul(
                    pg,
                    wc_sb[:, gc * 128 : (gc + 1) * 128],
                    xc[:, nb * 512 : (nb + 1) * 512],
                    start=True,
                    stop=True,
                )
                t0 = nb * (512 // BL)
                src = pg.rearrange("p (t b) -> p t b", b=BL)
                for q in range(2):  # small pieces: never head-of-line-block the scan
                    nc.vector.tensor_copy(
                        igbuf[:, t0 + 8 * q : t0 + 8 * (q + 1), gc * BL : (gc + 1) * BL],
                        src[:, 8 * q : 8 * (q + 1), :],
                    )
        return igbuf

    def ig_slice(igbuf, t, lo, hi):
        return igbuf[:, t, lo:hi].rearrange("p (c b) -> p c b", b=BL)

    P = [None, None]  # in-flight psum tiles {t%2: (P_r, P_z, P_n)}

    def preload(igbuf, t):
        """identity / b_n preload matmuls for P(t) (run early, off-chain)."""
        P_r = prz.tile([128, 2, BL], F32, tag="pr", name="P_r")
        P_z = prz.tile([128, 2, BL], F32, tag="pz", name="P_z")
        P_n = pn.tile([128, 2, BL], F32, tag="pn", name="P_n")
        nc.tensor.matmul(P_r, ident, ig_slice(igbuf, t % TC, 0, 64), start=True, stop=False)
        nc.tensor.matmul(P_z, ident, ig_slice(igbuf, t % TC, 64, 128), start=True, stop=False)
        nc.tensor.matmul(P_n, bnl_sb, sel_sb, start=True, stop=False)
        P[t % 2] = (P_r, P_z, P_n)

    # gate-group -> whh gate chunks; r first in the nzc pass so Sig(r) can
    # start earliest
    GB = (("r", 0, (0, 1)), ("n", 2, (4, 5)), ("z", 1, (2, 3)))

    def half_mms(t, rhs_t, is_last):
        """Accumulate W_hh @ rhs into P(t+1); the nzc pass (is_last) stops."""
        tiles = P[(t + 1) % 2]
        for gi, gcs in ((2, (4, 5)), (0, (0, 1)), (1, (2, 3))):
            for kc in range(2):
                for i, gc in enumerate(gcs):
                    nc.tensor.matmul(
                        tiles[gi][:, i, :],
                        whh_sb[:, kc, gc * 128 : (gc + 1) * 128],
                        rhs_t[:, kc, :],
                        start=False,
                        stop=(is_last and kc == 1),
                        skip_group_check=True,
                    )

    def ew_step(igbuf, t):
        P_r, P_z, P_n = P[t % 2]
        h_in = hT[0]
        # z path on ACT + Pool (off the critical chain)
        zc = ew.tile([128, 2, BL], F32, tag="zc", name="zc")  # 1-z
        nc.scalar.activation(zc, P_z, Sig, scale=-1.0)
        z_t = ew.tile([128, 2, BL], F32, tag="z", name="z_t")
        nc.scalar.activation(z_t, P_z, Sig)
        w_b = ew.tile([128, 2, BL], BF16, tag="wb", name="w_b")
        nc.gpsimd.tensor_mul(w_b, z_t, h_in)
        w_f = ew.tile([128, 2, BL], F32, tag="wf", name="w_f")
        nc.gpsimd.tensor_mul(w_f, z_t, h_in)
        # critical chain, all on DVE:
        #   sp = sig_poly(P_r); t2n = clip01(sp+.5)*P_n; tq = tanh((t2n+ig)/4)
        #   d1 = double(tq); d2 = double(d1); nzc = clip(d2,±1)*zc
        flat = lambda ap: ap.rearrange("p c b -> p (c b)")
        sp = ew.tile([128, 2 * BL], F32, tag="sp", name="sp")
        nc.vector._custom_dve(
            ODD5_OP, out=sp, in0=flat(P_r), s0=SIG5[0], s1=SIG5[1], imm2=SIG5[2]
        )
        t2n = ew.tile([128, 2 * BL], F32, tag="t2n", name="t2n")
        nc.vector._custom_dve(
            SIGCM_OP, out=t2n, in0=sp, in1=flat(P_n), s0=0.0, s1=0.0, imm2=0.5
        )
        tq = ew.tile([128, 2 * BL], F32, tag="tq", name="tq")
        nc.vector._custom_dve(
            ODD5S_OP,
            out=tq,
            in0=t2n,
            in1=igbuf[:, t % TC, 128:192],
            s0=T4C[0],
            s1=T4C[1],
            imm2=T4C[2],
        )
        d1 = ew.tile([128, 2 * BL], F32, tag="d1", name="d1")
        nc.vector._custom_dve(
            ODD5_OP, out=d1, in0=tq, s0=D1C[0], s1=D1C[1], imm2=D1C[2]
        )
        d2 = ew.tile([128, 2 * BL], F32, tag="d2", name="d2")
        nc.vector._custom_dve(
            ODD5_OP, out=d2, in0=d1, s0=D2C[0], s1=D2C[1], imm2=D2C[2]
        )
        nzc_b = ew.tile([128, 2, BL], BF16, tag="nzcb", name="nzc_b")
        nc.vector._custom_dve(
            TANHCM_OP, out=flat(nzc_b), in0=flat(zc), in1=d2, s0=1.0, s1=-1.0, imm2=0.0
        )
        # f32 state (off-chain)
        nzc_f = ew.tile([128, 2, BL], F32, tag="nzcf", name="nzc_f")
        nc.vector._custom_dve(
            TANHCM_OP, out=flat(nzc_f), in0=flat(zc), in1=d2, s0=1.0, s1=-1.0, imm2=0.0
        )
        hT_new = state.tile([128, 2, BL], F32, tag="h32", name="hT_new")
        nc.gpsimd.tensor_add(hT_new, nzc_f, w_f)
        hT[0] = hT_new
        return w_b, nzc_b

    xc0 = load_x(0)
    igbufs = {0: gemm_ig(0, xc0)}
    preload(igbufs[0], 0)
    half_mms(-1, w_b0, False)  # zero w-contribution for step 0 (sets nothing)
    prev = (w_b0, w_b0)  # (w_b, nzc_b) producing P(0): h(-1)=0 so both zero
    for tg in range(T_total):
        c = tg // TC
        if tg % TC == 4 and c + 1 < n_chunks:
            xc_n = load_x(c + 1)
            igbufs[c + 1] = gemm_ig(c + 1, xc_n)
            igbufs.pop(c - 1, None)
        igbuf = igbufs[c]
        # finish P(t): nzc-pass with nzc_b(t-1)
        half_mms(tg - 1, prev[1], True)
        if tg + 1 < T_total:
            preload(igbufs[(tg + 1) // TC], tg + 1)
        w_b, nzc_b = ew_step(igbuf, tg)
        if tg + 1 < T_total:
            half_mms(tg, w_b, False)  # w-pass for P(t+1), during tanh window
        prev = (w_b, nzc_b)

    # ---- output head: outT[o, b] = W_out @ h + b_out ----
    po_full = pgemm.tile([128, 512], F32, tag="pg")
    po = po_full[0:OUT, 0:BL]
    for kc in range(2):
        nc.tensor.matmul(
            po,
            wo_sb[:, kc, :],
            hT[0][:, kc, :],
            start=(kc == 0),
            stop=(kc == 1),
            skip_group_check=True,
        )
    osb = ew.tile([OUT, BL], F32, tag="osb")
    nc.vector.tensor_scalar(
        out=osb, in0=po, scalar1=bo_sb, scalar2=None, op0=mybir.AluOpType.add
    )
    nc.sync.dma_start(out=aps["outT"], in_=osb)


def build_nc(T_total=T):
    key = T_total
    if key in _nc_cache:
        return _nc_cache[key]
    nc = bacc.Bacc("TRN2", target_bir_lowering=False, debug=False, num_devices=N_CORES)
    aps = {
        "xT": nc.dram_tensor("xT", [IN, T_total, BL], BF16, kind="ExternalInput").ap(),
        "whhT": nc.dram_tensor("whhT", [H, G3], BF16, kind="ExternalInput").ap(),
        "wcT": nc.dram_tensor("wcT", [IN + 1, G3], BF16, kind="ExternalInput").ap(),
        "bnl": nc.dram_tensor("bnl", [2, 128], BF16, kind="ExternalInput").ap(),
        "sel": nc.dram_tensor("sel", [2, 2, BL], BF16, kind="ExternalInput").ap(),
        "ident": nc.dram_tensor("ident", [128, 128], BF16, kind="ExternalInput").ap(),
        "woT": nc.dram_tensor("woT", [H, OUT], F32, kind="ExternalInput").ap(),
        "bob": nc.dram_tensor("bob", [OUT, 1], F32, kind="ExternalInput").ap(),
        "outT": nc.dram_tensor("outT", [OUT, BL], F32, kind="ExternalOutput").ap(),
    }
    from contextlib import ExitStack

    with tile.TileContext(nc) as tc:
        with ExitStack() as es:
            _emit(es, tc, aps, T_total)
    nc.compile()
    _nc_cache[key] = (nc, aps)
    return nc, aps


def host_prep(x, W_in, b_in, W_ih, W_hh, b_ih, b_n, W_out, b_out, T_total=T):
    import ml_dtypes

    x = np.asarray(x, np.float32)
    f8 = np.float64
    W_c = (np.asarray(W_ih, f8) @ np.asarray(W_in, f8)).astype(np.float32)  # [3H, IN]
    b_c = (np.asarray(W_ih, f8) @ np.asarray(b_in, f8) + np.asarray(b_ih, f8)).astype(
        np.float32
    )
    whhT = np.ascontiguousarray(np.asarray(W_hh, np.float32).T).astype(
        ml_dtypes.bfloat16
    )  # [H, 3H]
    wcT = np.ascontiguousarray(np.vstack([W_c.T, b_c[None, :]])).astype(
        ml_dtypes.bfloat16
    )  # [IN+1, 3H]
    bn = np.asarray(b_n, np.float32)
    bnl = np.ascontiguousarray(bn.reshape(2, 128)).astype(
        ml_dtypes.bfloat16
    )  # K=2 lhsT: row c = b_n chunk c
    sel = np.zeros((2, 2, BL), ml_dtypes.bfloat16)  # rhs selector
    sel[0, 0, :] = 1.0
    sel[1, 1, :] = 1.0
    ident = np.eye(128, dtype=np.float32).astype(ml_dtypes.bfloat16)
    woT = np.ascontiguousarray(np.asarray(W_out, np.float32).T)  # [H, OUT]
    bob = np.asarray(b_out, np.float32).reshape(OUT, 1)

    shared = {
        "whhT": whhT,
        "wcT": wcT,
        "bnl": bnl,
        "sel": sel,
        "ident": ident,
        "woT": woT,
        "bob": bob,
    }
    in_maps = []
    for c in range(N_CORES):
        xc = x[c * BL : (c + 1) * BL, :T_total, :]  # [BL, T_total, IN]
        xTc = np.ascontiguousarray(xc.transpose(2, 1, 0)).astype(
            ml_dtypes.bfloat16
        )  # [IN, T_total, BL]
        in_maps.append({"xT": xTc, **shared})
    return in_maps


def kernel(x, W_in, b_in, W_ih, W_hh, b_ih, b_n, W_out, b_out):
    nc, _ = build_nc()
    in_maps = host_prep(x, W_in, b_in, W_ih, W_hh, b_ih, b_n, W_out, b_out)
    res = run_bass_kernel_spmd(nc, in_maps, core_ids=list(range(N_CORES)))
    out = np.concatenate(
        [res.results[c]["outT"].T for c in range(N_CORES)], axis=0
    )  # [B, OUT]
    return np.ascontiguousarray(out.astype(np.float32))


# revision 26
# speedup vs baseline: 1.0491x; 1.0491x over previous
"""GRU sequence model kernel for Trainium2 (8 NeuronCores, data-parallel).

Math (per reference):
  u  = x @ W_in.T + b_in              [B,T,H]
  ig = u @ W_ih.T + b_ih              [B,T,3H]   (folded: ig = x@W_c.T + b_c,
                                       with b_c as an extra K-row of the GEMM)
  scan over T:  hg = h @ W_hh.T
                r = sig(ig_r+hg_r); z = sig(ig_z+hg_z)
                n = tanh(ig_n + r*(hg_n + b_n)); h' = n + z*(h-n)
  out = h_T @ W_out.T + b_out         [B,OUT]

Sharding: B=256 split 32/core across 8 cores; weights replicated; T scan local.

Device layout is feature-on-partitions ("transposed"):
  state  hT  [128, 2, BL] f32   (h chunk c*128.., BL batch)
  psum   P_r/P_z/P_n [128, 2, BL] preloaded with ig_rz via identity matmuls /
         b_n via a K=2 selector matmul, then accumulating W_hh matmuls.

Scan recurrence is split-state: h' = w + nzc with w = z*h (available right
after Sig(z), early in the step) and nzc = (1-z)*n (available only after
tanh).  All W_hh matmuls accumulate the two addends separately, so the
w-pass runs during the tanh window and only the nzc-pass (12 small matmuls)
sits on the serial critical path:
  nzc_b -> PE nzc-pass (r group first) -> Sig(r) -> t2=r*P_n -> npre ->
  tanh -> nzc_b
"""

import sys

sys.path.insert(0, "/opt/trn_rl_repo")

import numpy as np

import concourse.bacc as bacc
import concourse.tile as tile
from concourse import mybir
from concourse.bass_utils import run_bass_kernel_spmd

# ---------------------------------------------------------------------------
# Custom DVE ops: polynomial sigmoid/tanh so the whole scan chain runs on the
# DVE with no ACT round-trips (each ACT op costs ~450ns: 185ns SBUF-access
# bubble in busy time plus another 185ns before its semaphore fires).
# Registered through the documented dve_ops extension point (OPS +
# _SUB_OPCODE_FOR_NAME + CUSTOM_DVE_SPECS) with shas computed at import.
# ---------------------------------------------------------------------------
from concourse import dve_ops as _dvo
from concourse.dve_spec import C0, C1, C2, One, Spec, Src0, Src1, Zero, maxx, minn, sq
from concourse.dve_table_gen import dve_ver_for
from concourse.dve_uop import DveOpSpec
from concourse.dve_spec import _has_src1 as _spec_has_src1, lower as _spec_lower


def _register_dve_op(name, spec, subdim=False):
    if name in _dvo._SUB_OPCODE_FOR_NAME:
        return next(op for op in _dvo.OPS if op.name == name)
    ver = dve_ver_for("TRN2")
    row = _dvo._CUSTOM_DVE_ROW_BASE + len(_dvo.OPS)
    lowered = DveOpSpec(
        name=name, opcode=row, uops=_spec_lower(spec, ver=ver),
        rd1_en=_spec_has_src1(spec),
    )
    op = _dvo.DveOp(name, spec, subdim=subdim, uops_sha={ver: lowered.sha(ver)})
    _dvo.OPS.append(op)
    _dvo._SUB_OPCODE_FOR_NAME[name] = row
    _dvo.CUSTOM_DVE_SPECS[name] = spec
    return op


def _mk_ops():
    # out = in0*(c0 + u*(c1 + u*c2)), u = in0^2  — odd quintic
    _u0 = sq(Src0)
    ODD5 = _register_dve_op(
        "GRU_ODD5_ANT",
        Spec(
            body=Src0 * (C0 + _u0 * (C1 + _u0 * C2)),
            reference=lambda in0, in1, c0, c1, c2: (
                lambda u: in0 * (c0 + u * (c1 + u * c2))
            )(in0 * in0),
        ),
    )
    # out = x*(c0 + u*(c1 + u*c2)), x = in0+in1, u = x^2 — odd quintic of a sum
    _x = Src0 + Src1
    _ux = sq(_x)
    ODD5S = _register_dve_op(
        "GRU_ODD5SUM_ANT",
        Spec(
            body=_x * (C0 + _ux * (C1 + _ux * C2)),
            reference=lambda in0, in1, c0, c1, c2: (
                lambda x: (lambda u: x * (c0 + u * (c1 + u * c2)))(x * x)
            )(in0 + in1),
        ),
    )
    # out = clip(in0 + c2, 0, 1) * in1  — sigmoid shift/clamp fused with mul
    SIGCM = _register_dve_op(
        "GRU_SIGCM_ANT",
        Spec(
            body=maxx(minn(Src0 + C2, One), Zero) * Src1,
            reference=lambda in0, in1, c0, c1, c2: np.clip(in0 + c2, 0.0, 1.0) * in1,
        ),
    )
    # out = in0 * clip(in1, c1, c0)  — tanh clamp fused with mul; the clamped
    # operand is Src1 so the zc factor (early, cross-engine) sits in Src0 and
    # its wait lands on the EventSemaphore while the late same-engine dep
    # rides the attached wait slot.
    TANHCM = _register_dve_op(
        "GRU_TANHCM_ANT",
        Spec(
            body=Src0 * maxx(minn(Src1, C0), C1),
            reference=lambda in0, in1, c0, c1, c2: in0 * np.minimum(
                np.maximum(in1, c1), c0
            ),
        ),
    )
    return ODD5, ODD5S, SIGCM, TANHCM


ODD5_OP, ODD5S_OP, SIGCM_OP, TANHCM_OP = _mk_ops()

# polynomial constants (fit against the reference value ranges; see notes)
SIG5 = (0.2460386709429935, -0.016334294476830553, 0.0006460262005284434)
T4C = (0.24960876935340268, -0.004928416054628486, 7.916056459559794e-05)
D1C = (1.988672907492938, -1.7884419833362064, 0.9587148526678364)
D2C = (1.975424316113866, -1.6426980934515871, 0.6994047528181397)

B, T, IN, H, OUT = 256, 2048, 64, 256, 32
N_CORES = 8
BL = B // N_CORES  # 32 batch rows per core
TC = 64  # scan chunk length (steps per ig buffer)
G3 = 3 * H
F32 = mybir.dt.float32
BF16 = mybir.dt.bfloat16

_nc_cache = {}


def _emit(ctx, tc, aps, T_total):
    nc = tc.nc
    n_chunks = T_total // TC
    Sig = mybir.ActivationFunctionType.Sigmoid
    Tanh = mybir.ActivationFunctionType.Tanh
    Mult = mybir.AluOpType.mult
    Add = mybir.AluOpType.add

    singles = ctx.enter_context(tc.tile_pool(name="singles", bufs=1))
    xpool = ctx.enter_context(tc.tile_pool(name="xpool", bufs=2))
    igpool = ctx.enter_context(tc.tile_pool(name="igpool", bufs=2))
    ew = ctx.enter_context(tc.tile_pool(name="ew", bufs=16))
    state = ctx.enter_context(tc.tile_pool(name="state", bufs=4))
    prz = ctx.enter_context(tc.tile_pool(name="prz", bufs=2, space="PSUM"))
    pn = ctx.enter_context(tc.tile_pool(name="pn", bufs=2, space="PSUM"))
    pgemm = ctx.enter_context(tc.tile_pool(name="pgemm", bufs=2, space="PSUM"))

    # ---- weights into SBUF (once) ----
    whh_sb = singles.tile([128, 2, G3], BF16)  # [k, kc, g] : W_hh.T chunks
    nc.sync.dma_start(out=whh_sb, in_=aps["whhT"].rearrange("(c k) g -> k c g", k=128))
    wc_sb = singles.tile([IN + 1, G3], BF16)  # W_c.T with b_c as row IN
    nc.sync.dma_start(out=wc_sb, in_=aps["wcT"])
    bnl_sb = singles.tile([2, 128], BF16)  # b_n chunks as K=2 matmul lhsT
    nc.sync.dma_start(out=bnl_sb, in_=aps["bnl"])
    sel_sb = singles.tile([2, 2, BL], BF16)  # block selector rhs
    nc.sync.dma_start(out=sel_sb, in_=aps["sel"])
    ident = singles.tile([128, 128], BF16)
    nc.sync.dma_start(out=ident, in_=aps["ident"])
    wo_sb = singles.tile([128, 2, OUT], F32)  # W_out.T chunks [k, kc, o]
    nc.sync.dma_start(out=wo_sb, in_=aps["woT"].rearrange("(c k) o -> k c o", k=128))
    bo_sb = singles.tile([OUT, 1], F32)
    nc.sync.dma_start(out=bo_sb, in_=aps["bob"])

    # ---- state ----
    hT = [state.tile([128, 2, BL], F32, tag="h32", name="hT0")]
    nc.vector.memset(hT[0], 0.0)
    w_b0 = state.tile([128, 2, BL], BF16, tag="wb0", name="wb0")
    nc.vector.memset(w_b0, 0.0)  # w(t=0) = z*h(-1) = 0

    xT = aps["xT"]  # [IN, T_total, BL]

    def load_x(c):
        # row IN is the constant-1 row that multiplies the b_c row of wc_sb;
        # pool slots rotate round-robin so only the first bufs chunks memset it.
        xc = xpool.tile([IN + 1, TC * BL], BF16, tag="xc")
        nc.sync.dma_start(
            out=xc[0:IN, :],
            in_=xT[:, c * TC : (c + 1) * TC, :].rearrange("i t b -> i (t b)"),
        )
        if c < 2:
            nc.vector.memset(xc[IN : IN + 1, :], 1.0)
        return xc

    def gemm_ig(c, xc):
        """igbuf[p, t, gc*BL+b] = (W_c @ x + b_c)[g, t, b]"""
        igbuf = igpool.tile([128, TC, 6 * BL], BF16, tag="ig")
        nblk = TC * BL // 512
        for nb in range(nblk):
            for gc in range(6):
                pg = pgemm.tile([128, 512], F32, tag="pg")
                nc.tensor.matmul(
                    pg,
                    wc_sb[:, gc * 128 : (gc + 1) * 128],
                    xc[:, nb * 512 : (nb + 1) * 512],
                    start=True,
                    stop=True,
                )
                t0 = nb * (512 // BL)
                src = pg.rearrange("p (t b) -> p t b", b=BL)
                for q in range(2):  # small pieces: never head-of-line-block the scan
                    nc.vector.tensor_copy(
                        igbuf[:, t0 + 8 * q : t0 + 8 * (q + 1), gc * BL : (gc + 1) * BL],
                        src[:, 8 * q : 8 * (q + 1), :],
                    )
        return igbuf

    def ig_slice(igbuf, t, lo, hi):
        return igbuf[:, t, lo:hi].rearrange("p (c b) -> p c b", b=BL)

    P = [None, None]  # in-flight psum tiles {t%2: (P_r, P_z, P_n)}

    def preload(igbuf, t):
        """identity / b_n preload matmuls for P(t) (run early, off-chain)."""
        P_r = prz.tile([128, 2, BL], F32, tag="pr", name="P_r")
        P_z = prz.tile([128, 2, BL], F32, tag="pz", name="P_z")
        P_n = pn.tile([128, 2, BL], F32, tag="pn", name="P_n")
        nc.tensor.matmul(P_r, ident, ig_slice(igbuf, t % TC, 0, 64), start=True, stop=False)
        nc.tensor.matmul(P_z, ident, ig_slice(igbuf, t % TC, 64, 128), start=True, stop=False)
        nc.tensor.matmul(P_n, bnl_sb, sel_sb, start=True, stop=False)
        P[t % 2] = (P_r, P_z, P_n)

    # gate-group -> whh gate chunks; r first in the nzc pass so Sig(r) can
    # start earliest
    GB = (("r", 0, (0, 1)), ("n", 2, (4, 5)), ("z", 1, (2, 3)))

    def half_mms(t, rhs_t, is_last):
        """Accumulate W_hh @ rhs into P(t+1); the nzc pass (is_last) stops."""
        tiles = P[(t + 1) % 2]
        for gi, gcs in ((2, (4, 5)), (0, (0, 1)), (1, (2, 3))):
            for kc in range(2):
                for i, gc in enumerate(gcs):
                    nc.tensor.matmul(
                        tiles[gi][:, i, :],
                        whh_sb[:, kc, gc * 128 : (gc + 1) * 128],
                        rhs_t[:, kc, :],
                        start=False,
                        stop=(is_last and kc == 1),
                        skip_group_check=True,
                    )

    def ew_step(igbuf, t):
        P_r, P_z, P_n = P[t % 2]
        h_in = hT[0]
        # z path on ACT + Pool (off the critical chain)
        zc = ew.tile([128, 2, BL], F32, tag="zc", name="zc")  # 1-z
        nc.scalar.activation(zc, P_z, Sig, scale=-1.0)
        z_t = ew.tile([128, 2, BL], F32, tag="z", name="z_t")
        nc.scalar.activation(z_t, P_z, Sig)
        w_b = ew.tile([128, 2, BL], BF16, tag="wb", name="w_b")
        nc.gpsimd.tensor_mul(w_b, z_t, h_in)
        w_f = ew.tile([128, 2, BL], F32, tag="wf", name="w_f")
        nc.gpsimd.tensor_mul(w_f, z_t, h_in)
        # critical chain, all on DVE:
        #   sp = sig_poly(P_r); t2n = clip01(sp+.5)*P_n; tq = tanh((t2n+ig)/4)
        #   d1 = double(tq); d2 = double(d1); nzc = clip(d2,±1)*zc
        flat = lambda ap: ap.rearrange("p c b -> p (c b)")
        sp = ew.tile([128, 2 * BL], F32, tag="sp", name="sp")
        nc.vector._custom_dve(
            ODD5_OP, out=sp, in0=flat(P_r), s0=SIG5[0], s1=SIG5[1], imm2=SIG5[2]
        )
        t2n = ew.tile([128, 2 * BL], F32, tag="t2n", name="t2n")
        nc.vector._custom_dve(
            SIGCM_OP, out=t2n, in0=sp, in1=flat(P_n), s0=0.0, s1=0.0, imm2=0.5
        )
        npre = ew.tile([128, 2 * BL], F32, tag="npre", name="npre")
        nc.vector.tensor_add(npre, t2n, igbuf[:, t % TC, 128:192])
        n_t = ew.tile([128, 2 * BL], F32, tag="nt", name="n_t")
        nc.scalar.activation(n_t, npre, Tanh)
        # both nzc factors come from ACT (n_t, zc): a single dominated wait
        nzc_b = ew.tile([128, 2, BL], BF16, tag="nzcb", name="nzc_b")
        nc.vector.tensor_mul(flat(nzc_b), n_t, flat(zc))
        # f32 state (off-chain)
        nzc_f = ew.tile([128, 2, BL], F32, tag="nzcf", name="nzc_f")
        nc.vector.tensor_mul(flat(nzc_f), n_t, flat(zc))
        hT_new = state.tile([128, 2, BL], F32, tag="h32", name="hT_new")
        nc.gpsimd.tensor_add(hT_new, nzc_f, w_f)
        hT[0] = hT_new
        return w_b, nzc_b

    xc0 = load_x(0)
    igbufs = {0: gemm_ig(0, xc0)}
    preload(igbufs[0], 0)
    half_mms(-1, w_b0, False)  # zero w-contribution for step 0 (sets nothing)
    prev = (w_b0, w_b0)  # (w_b, nzc_b) producing P(0): h(-1)=0 so both zero
    for tg in range(T_total):
        c = tg // TC
        if tg % TC == 4 and c + 1 < n_chunks:
            xc_n = load_x(c + 1)
            igbufs[c + 1] = gemm_ig(c + 1, xc_n)
            igbufs.pop(c - 1, None)
        igbuf = igbufs[c]
        # finish P(t): nzc-pass with nzc_b(t-1)
        half_mms(tg - 1, prev[1], True)
        if tg + 1 < T_total:
            preload(igbufs[(tg + 1) // TC], tg + 1)
        w_b, nzc_b = ew_step(igbuf, tg)
        if tg + 1 < T_total:
            half_mms(tg, w_b, False)  # w-pass for P(t+1), during tanh window
        prev = (w_b, nzc_b)

    # ---- output head: outT[o, b] = W_out @ h + b_out ----
    po_full = pgemm.tile([128, 512], F32, tag="pg")
    po = po_full[0:OUT, 0:BL]
    for kc in range(2):
        nc.tensor.matmul(
            po,
            wo_sb[:, kc, :],
            hT[0][:, kc, :],
            start=(kc == 0),
            stop=(kc == 1),
            skip_group_check=True,
        )
    osb = ew.tile([OUT, BL], F32, tag="osb")
    nc.vector.tensor_scalar(
        out=osb, in0=po, scalar1=bo_sb, scalar2=None, op0=mybir.AluOpType.add
    )
    nc.sync.dma_start(out=aps["outT"], in_=osb)


def build_nc(T_total=T):
    key = T_total
    if key in _nc_cache:
        return _nc_cache[key]
    nc = bacc.Bacc("TRN2", target_bir_lowering=False, debug=False, num_devices=N_CORES)
    aps = {
        "xT": nc.dram_tensor("xT", [IN, T_total, BL], BF16, kind="ExternalInput").ap(),
        "whhT": nc.dram_tensor("whhT", [H, G3], BF16, kind="ExternalInput").ap(),
        "wcT": nc.dram_tensor("wcT", [IN + 1, G3], BF16, kind="ExternalInput").ap(),
        "bnl": nc.dram_tensor("bnl", [2, 128], BF16, kind="ExternalInput").ap(),
        "sel": nc.dram_tensor("sel", [2, 2, BL], BF16, kind="ExternalInput").ap(),
        "ident": nc.dram_tensor("ident", [128, 128], BF16, kind="ExternalInput").ap(),
        "woT": nc.dram_tensor("woT", [H, OUT], F32, kind="ExternalInput").ap(),
        "bob": nc.dram_tensor("bob", [OUT, 1], F32, kind="ExternalInput").ap(),
        "outT": nc.dram_tensor("outT", [OUT, BL], F32, kind="ExternalOutput").ap(),
    }
    from contextlib import ExitStack

    with tile.TileContext(nc) as tc:
        with ExitStack() as es:
            _emit(es, tc, aps, T_total)
    nc.compile()
    _nc_cache[key] = (nc, aps)
    return nc, aps


def host_prep(x, W_in, b_in, W_ih, W_hh, b_ih, b_n, W_out, b_out, T_total=T):
    import ml_dtypes

    x = np.asarray(x, np.float32)
    f8 = np.float64
    W_c = (np.asarray(W_ih, f8) @ np.asarray(W_in, f8)).astype(np.float32)  # [3H, IN]
    b_c = (np.asarray(W_ih, f8) @ np.asarray(b_in, f8) + np.asarray(b_ih, f8)).astype(
        np.float32
    )
    whhT = np.ascontiguousarray(np.asarray(W_hh, np.float32).T).astype(
        ml_dtypes.bfloat16
    )  # [H, 3H]
    wcT = np.ascontiguousarray(np.vstack([W_c.T, b_c[None, :]])).astype(
        ml_dtypes.bfloat16
    )  # [IN+1, 3H]
    bn = np.asarray(b_n, np.float32)
    bnl = np.ascontiguousarray(bn.reshape(2, 128)).astype(
        ml_dtypes.bfloat16
    )  # K=2 lhsT: row c = b_n chunk c
    sel = np.zeros((2, 2, BL), ml_dtypes.bfloat16)  # rhs selector
    sel[0, 0, :] = 1.0
    sel[1, 1, :] = 1.0
    ident = np.eye(128, dtype=np.float32).astype(ml_dtypes.bfloat16)
    woT = np.ascontiguousarray(np.asarray(W_out, np.float32).T)  # [H, OUT]
    bob = np.asarray(b_out, np.float32).reshape(OUT, 1)

    shared = {
        "whhT": whhT,
        "wcT": wcT,
        "bnl": bnl,
        "sel": sel,
        "ident": ident,
        "woT": woT,
        "bob": bob,
    }
    in_maps = []
    for c in range(N_CORES):
        xc = x[c * BL : (c + 1) * BL, :T_total, :]  # [BL, T_total, IN]
        xTc = np.ascontiguousarray(xc.transpose(2, 1, 0)).astype(
            ml_dtypes.bfloat16
        )  # [IN, T_total, BL]
        in_maps.append({"xT": xTc, **shared})
    return in_maps


def kernel(x, W_in, b_in, W_ih, W_hh, b_ih, b_n, W_out, b_out):
    nc, _ = build_nc()
    in_maps = host_prep(x, W_in, b_in, W_ih, W_hh, b_ih, b_n, W_out, b_out)
    res = run_bass_kernel_spmd(nc, in_maps, core_ids=list(range(N_CORES)))
    out = np.concatenate(
        [res.results[c]["outT"].T for c in range(N_CORES)], axis=0
    )  # [B, OUT]
    return np.ascontiguousarray(out.astype(np.float32))
